# revision 18
# baseline (speedup 1.0000x reference)
"""Trainium2 Bass kernel for nn_BlockLevelRouter (gnn_message_passing).

Strategy
--------
Pure data parallel over B=2048 across 8 cores (256 b each). Key insights:

* block_active is all-ones and cancer_type has only 32 values, so the per-b
  edge weights take at most ~32 distinct vectors. Message passing
  `scatter_add(edge_w * h_route[src])` collapses to a dense 101x101 adjacency
  matmul with one matrix per distinct weight vector ("type").
* Batch elements are re-ordered into type-grouped "slots" with identical
  per-type quotas on every core, so one SPMD program (adjacency choice baked
  into the instruction stream) serves all 8 cores.
* Activations are kept feature-major [h, token]; the only token-contraction
  (adjacency matmul) uses h_route as the stationary operand so NO on-chip
  transposes are needed anywhere. Host transposes input/output layouts.
* Matmuls run in float32r (1 cycle/row at free-dim>=256) or bf16.
* LayerNorm: mean/sumsq via ones-column matmuls; the mean subtraction is
  folded into the FFN matmul as a rank-1 (-c1 x mu*rs) term; rsqrt via
  ACT Ln+Exp so the whole kernel uses ONE activation table set.
* ELU(z) = min(relu(z), exp(z)-1) built from ACT Exp + DVE tensor_scalar +
  scalar_tensor_tensor.  sigmoid(z) = 1/(1+exp(-z)) via ACT Exp + DVE recip.
"""

import os
import sys

sys.path.insert(0, "/opt/trn_rl_repo")

from contextlib import ExitStack

import numpy as np
import ml_dtypes  # noqa: F401  (bf16 numpy dtype)

import concourse.bacc as bacc
import concourse.bass as bass
import concourse.tile as tile
from concourse import mybir
from concourse.bass_utils import run_bass_kernel_spmd

F32 = mybir.dt.float32
F32R = mybir.dt.float32r
BF16 = mybir.dt.bfloat16
BF16_NP = ml_dtypes.bfloat16
AF = mybir.ActivationFunctionType
OP = mybir.AluOpType

# Problem constants (hardcoded per the task contract).
B, NB, H, E, NCT, NL = 2048, 101, 256, 656, 32, 2
K1 = 2 * H  # 512
LN_EPS = 1e-5
NCORES = 8
TB = 5            # batch elements per tile
TN = TB * NB      # 505 tokens per tile
TNP = 512         # padded tile width for SBUF elementwise ops


# --------------------------------------------------------------------------
# Host-side planning
# --------------------------------------------------------------------------

def _plan_slots(cancer_type, block_active, edge_src, edge_dst, edge_structure,
                ct_w, ncores=NCORES):
    """Group batch elements by distinct edge-weight vector; build per-core
    slot schedules with identical per-type quotas, and the adjacency tables.
    """
    sig = 1.0 / (1.0 + np.exp(-edge_structure.astype(np.float64)))
    sig = sig.astype(np.float32)
    wtab = sig[None, :] * ct_w  # (NCT, E)

    if bool(block_active.all()):
        type_of_b = cancer_type.astype(np.int64)
        wvec = wtab  # (NCT, E)
        nun = NCT
    else:
        # General path: distinct (cancer_type, active-pattern) combos.
        act = block_active
        am = (act[:, edge_src] & act[:, edge_dst]).astype(np.float32)  # (B,E)
        wb = wtab[cancer_type] * am                                    # (B,E)
        wvec, type_of_b = np.unique(wb, axis=0, return_inverse=True)
        nun = wvec.shape[0]
        assert nun <= 128, f"too many distinct weight rows: {nun}"

    # Assign b's of each type round-robin across cores.
    per_core_b = [[] for _ in range(ncores)]  # list of (slot-ordered) b per core
    slot_type = []
    for u in range(nun):
        bs = np.nonzero(type_of_b == u)[0]
        if len(bs) == 0:
            continue
        q = (len(bs) + ncores - 1) // ncores
        chunks = [bs[i::ncores] for i in range(ncores)]
        for i in range(ncores):
            lst = list(chunks[i])
            while len(lst) < q:  # pad with a duplicate (output discarded)
                lst.append(int(bs[0]))
            per_core_b[i].extend(lst)
        slot_type.extend([u] * q)

    # Pad slot count to a multiple of TB.
    while len(slot_type) % TB != 0:
        slot_type.append(slot_type[-1])
        for i in range(ncores):
            per_core_b[i].append(per_core_b[i][-1])
    Q = len(slot_type)
    slot_type = np.asarray(slot_type, dtype=np.int64)
    per_core_b = np.asarray(per_core_b, dtype=np.int64)  # (ncores, Q)

    # Adjacency tables AT[u][s, d] = sum_e 1[src=s,dst=d] * wvec[u][e]
    AT = np.zeros((nun, NB, NB), dtype=np.float32)
    for u in range(nun):
        np.add.at(AT[u], (edge_src.astype(np.int64), edge_dst.astype(np.int64)),
                  wvec[u])

    # Runs of equal type (for edge_w broadcast DMAs).
    runs = []
    r0 = 0
    for j in range(1, Q + 1):
        if j == Q or slot_type[j] != slot_type[r0]:
            runs.append((int(slot_type[r0]), r0, j))
            r0 = j
    return dict(Q=Q, nun=nun, slot_type=slot_type, per_core_b=per_core_b,
                wvec=wvec, AT=AT, runs=runs)


# --------------------------------------------------------------------------
# Device program
# --------------------------------------------------------------------------

def _r(ap):
    return ap.bitcast(F32R)


def build_program(plan, params, num_cores=NCORES):
    """Build the SPMD Bass program. `params` carries host-precomputed weights.

    Returns (nc, in_names) where in_names lists per-core input tensor names.
    """
    Q = plan["Q"]
    nun = plan["nun"]
    slot_type = plan["slot_type"]
    runs = plan["runs"]
    NT = Q // TB
    TT = Q * NB

    pb_nz = bool(np.any(params["proj_b"] != 0.0))
    d1_nz = bool(np.any(params["d1"] != 0.0))
    feat = os.environ.get("KFEAT", "all")
    def on(name):
        return feat == "all" or name in feat.split(",")

    nc = bacc.Bacc("TRN2", target_bir_lowering=False, debug=False,
                   num_devices=num_cores)

    def din(name, shape, dt):
        return nc.dram_tensor(name, list(shape), dt, kind="ExternalInput").ap()

    def dout(name, shape, dt):
        return nc.dram_tensor(name, list(shape), dt, kind="ExternalOutput").ap()

    xfm = din("xfm", (H, TT + TNP), F32)
    ATm = din("ATm", (NB, nun * NB), BF16)
    wvd = din("wvd", (nun, E), F32)
    pWd = din("pWd", (H, H), F32)
    gWxd = din("gWxd", (H, H), F32)
    gWmd = din("gWmd", (H, H), BF16)
    W1gd = din("W1gd", (NL, H, K1), F32)
    W2d = din("W2d", (NL, K1, H), BF16)
    negc1d = din("negc1d", (1, NL * K1), F32)
    gbnd = din("gbnd", (128, 2), F32)      # -gate_b as [128, km] columns
    b2cd = din("b2cd", (128, NL * 2), F32)  # b2 as [128, l*2+km] columns
    identd = din("identd", (128, 128), F32)
    onescd = din("onescd", (128, 1), F32)
    onesrd = din("onesrd", (1, 128), F32)
    if pb_nz:
        pbd = din("pbd", (1, H), F32)
    if d1_nz:
        d1d = din("d1d", (1, NL * K1), F32)
        onestnd = din("onestnd", (1, TNP), F32)

    xout = dout("xout", (H, TT), F32)
    ewout = dout("ewout", (Q, E), F32)

    with tile.TileContext(nc) as tc, ExitStack() as ctx:
        cp = ctx.enter_context(tc.tile_pool(name="consts", bufs=1))
        wp = ctx.enter_context(tc.tile_pool(name="work", bufs=2))
        pp = ctx.enter_context(tc.tile_pool(name="psum", bufs=2, space="PSUM"))

        pW_sb = [cp.tile([128, H], F32, name=f"pW{i}") for i in range(2)]
        for i in range(2):
            nc.sync.dma_start(_r(pW_sb[i][:, :]), _r(pWd[i * 128:(i + 1) * 128, :]))
        gWx_sb = [cp.tile([128, H], F32, name=f"gWx{i}") for i in range(2)]
        for i in range(2):
            nc.sync.dma_start(_r(gWx_sb[i][:, :]), _r(gWxd[i * 128:(i + 1) * 128, :]))
        gWm_sb = [cp.tile([128, H], BF16, name=f"gWm{i}") for i in range(2)]
        for i in range(2):
            nc.sync.dma_start(gWm_sb[i][:, :], gWmd[i * 128:(i + 1) * 128, :])
        W1g_sb = [[cp.tile([128, K1], F32, name=f"W1g{l}{i}") for i in range(2)]
                  for l in range(NL)]
        for l in range(NL):
            for i in range(2):
                nc.sync.dma_start(_r(W1g_sb[l][i][:, :]),
                                  _r(W1gd[l, i * 128:(i + 1) * 128, :]))
        W2_sb = [[cp.tile([128, H], BF16, name=f"W2{l}{i}") for i in range(4)]
                 for l in range(NL)]
        for l in range(NL):
            for i in range(4):
                nc.sync.dma_start(W2_sb[l][i][:, :],
                                  W2d[l, i * 128:(i + 1) * 128, :])
        AT_sb = cp.tile([NB, nun * NB], BF16, name="AT_sb")
        nc.sync.dma_start(AT_sb[:, :], ATm[:, :])
        wv_sb = cp.tile([nun, E], F32, name="wv_sb")
        nc.sync.dma_start(wv_sb[:, :], wvd[:, :])
        negc1 = cp.tile([1, NL * K1], F32, name="negc1")
        nc.sync.dma_start(_r(negc1[:, :]), _r(negc1d[:, :]))
        gbn = cp.tile([128, 2], F32, name="gbn")
        nc.sync.dma_start(gbn[:, :], gbnd[:, :])
        b2c = cp.tile([128, NL * 2], F32, name="b2c")
        nc.sync.dma_start(b2c[:, :], b2cd[:, :])
        ident = cp.tile([128, 128], F32, name="ident")
        nc.sync.dma_start(_r(ident[:, :]), _r(identd[:, :]))
        onesc = cp.tile([128, 1], F32, name="onesc")
        nc.sync.dma_start(_r(onesc[:, :]), _r(onescd[:, :]))
        onesr = cp.tile([1, 128], F32, name="onesr")
        nc.sync.dma_start(_r(onesr[:, :]), _r(onesrd[:, :]))
        epsc = cp.tile([1, 1], F32, name="epsc")
        nc.gpsimd.memset(epsc[:, :], LN_EPS)
        if pb_nz:
            pb_sb = cp.tile([1, H], F32, name="pb_sb")
            nc.sync.dma_start(_r(pb_sb[:, :]), _r(pbd[:, :]))
        if d1_nz:
            d1_sb = cp.tile([1, NL * K1], F32, name="d1_sb")
            nc.sync.dma_start(_r(d1_sb[:, :]), _r(d1d[:, :]))
            onestn = cp.tile([1, TNP], F32, name="onestn")
            nc.sync.dma_start(_r(onestn[:, :]), _r(onestnd[:, :]))

        # edge_w output: broadcast wvec rows into slot runs (DRAM->DRAM).
        for (u, r0, r1) in (runs if on("ew") else []):
            nc.sync.dma_start(
                ewout[r0:r1, :],
                wvd[u:u + 1, :].broadcast_to((r1 - r0, E)),
            )

        mm = nc.tensor.matmul

        def layernorm_and_ffn(it, l, xin):
            """xin: [2][128, TN(within TNP)] f32 SBUF. Returns x_next tiles
            (SBUF f32) holding xin + FFN(LN(xin)); for the last layer the
            caller DMAs them out."""
            # squares (gpsimd, SBUF only)
            sq = [wp.tile([128, TNP], F32, name=f"sq{it}_{l}_{h}", tag=f"sq{h}")
                  for h in range(2)]
            for h in range(2):
                nc.gpsimd.tensor_tensor(_r(sq[h][:, :TNP]), xin[h][:, :TNP],
                                        xin[h][:, :TNP], OP.mult)
            # stats: sum x, sum x^2 (separate psum tiles, both base-0)
            pst1 = pp.tile([1, TNP], F32, name=f"pst1{it}_{l}", tag="A", bufs=3)
            pst2 = pp.tile([1, TNP], F32, name=f"pst2{it}_{l}", tag="A", bufs=3)
            mm(pst1[:, :], _r(onesc[:, :]), _r(xin[0][:, :TNP]),
               start=True, stop=False)
            mm(pst1[:, :], _r(onesc[:, :]), _r(xin[1][:, :TNP]),
               start=False, stop=True)
            mm(pst2[:, :], _r(onesc[:, :]), _r(sq[0][:, :TNP]),
               start=True, stop=False)
            mm(pst2[:, :], _r(onesc[:, :]), _r(sq[1][:, :TNP]),
               start=False, stop=True)
            st = wp.tile([1, TNP], F32, name=f"st{it}_{l}", tag="st")
            st2 = wp.tile([1, TNP], F32, name=f"st2{it}_{l}", tag="st2")
            nc.vector.tensor_copy(st[0:1, :], pst1[:, :])
            nc.vector.tensor_copy(st2[0:1, :], pst2[:, :])
            # q = s1^2/H ; var' = s2 - q ; rs = exp(-0.5*ln(var'/H + eps))
            q = wp.tile([1, TNP], F32, name=f"q{it}_{l}", tag="nq")
            nc.vector.scalar_tensor_tensor(q[:, :TNP], st[0:1, :TNP], 1.0 / H,
                                           st[0:1, :TNP], OP.mult, OP.mult)
            vp = wp.tile([1, TNP], F32, name=f"vp{it}_{l}", tag="nv")
            nc.vector.tensor_tensor(vp[:, :TNP], st2[0:1, :TNP], q[:, :TNP],
                                    OP.subtract)
            lnv = wp.tile([1, TNP], F32, name=f"lnv{it}_{l}", tag="nl")
            nc.scalar.activation(lnv[:, :TNP], vp[:, :TNP], AF.Ln,
                                 bias=epsc[:, :], scale=1.0 / H)
            rs = wp.tile([1, TNP], F32, name=f"rs{it}_{l}", tag="nr")
            nc.scalar.activation(_r(rs[:, :TNP]), lnv[:, :TNP], AF.Exp, scale=-0.5)
            # murs = (s1/H)*rs
            murs = wp.tile([1, TNP], F32, name=f"murs{it}_{l}", tag="nm")
            nc.vector.scalar_tensor_tensor(_r(murs[:, :TNP]), st[0:1, :TNP],
                                           1.0 / H, rs[:, :TNP],
                                           OP.mult, OP.mult)
            # replicate rs across partitions; xn = xin * rs
            prs = pp.tile([128, TNP], F32, name=f"prs{it}_{l}", tag="B", bufs=4)
            mm(prs[:, :], _r(onesr[:, :]), _r(rs[:, :TNP]))
            xn = [wp.tile([128, TNP], F32, name=f"xn{it}_{l}_{h}", tag=f"xn{h}")
                  for h in range(2)]
            for h in range(2):
                nc.vector.tensor_tensor(_r(xn[h][:, :TNP]), xin[h][:, :TNP],
                                        prs[:, :], OP.mult)
            # FFN1 + fused mean-correction; ELU -> g1 (bf16)
            g1 = []
            for mj in range(4):
                ph = pp.tile([128, TNP], F32, name=f"ph{it}_{l}_{mj}",
                             tag="B", bufs=4)
                ms = slice(mj * 128, (mj + 1) * 128)
                mm(ph[:, :], _r(W1g_sb[l][0][:, ms]), _r(xn[0][:, :TNP]),
                   start=True, stop=False)
                mm(ph[:, :], _r(W1g_sb[l][1][:, ms]), _r(xn[1][:, :TNP]),
                   start=False, stop=False)
                cs = slice(l * K1 + mj * 128, l * K1 + (mj + 1) * 128)
                mm(ph[:, :], _r(negc1[:, cs]), _r(murs[:, :TNP]),
                   start=False, stop=not d1_nz)
                if d1_nz:
                    mm(ph[:, :], _r(d1_sb[:, cs]), _r(onestn[:, :TNP]),
                       start=False, stop=True)
                e = wp.tile([128, TNP], BF16, name=f"e{it}_{l}_{mj}", tag="fe")
                nc.scalar.activation(e[:, :TNP], ph[:, :], AF.Exp)
                em1 = wp.tile([128, TNP], BF16, name=f"em1{it}_{l}_{mj}",
                              tag="fm")
                nc.vector.tensor_scalar(em1[:, :], e[:, :], -1.0, None, OP.add)
                g = wp.tile([128, TNP], BF16, name=f"g1{it}_{l}_{mj}",
                            tag=f"g1{mj}")
                nc.vector.scalar_tensor_tensor(g[:, :TNP], ph[:, :], 0.0,
                                               em1[:, :TNP], OP.max, OP.min)
                g1.append(g)
            # FFN2 with residual preloaded via identity matmul
            xnext = []
            for km in range(2):
                px = pp.tile([128, TNP], F32, name=f"px{it}_{l}_{km}",
                             tag="B", bufs=4)
                mm(px[:, :], _r(ident[:, :]), _r(xin[km][:, :TNP]),
                   start=True, stop=False)
                for ci in range(4):
                    ks = slice(km * 128, (km + 1) * 128)
                    mm(px[:, :], W2_sb[l][ci][:, ks], g1[ci][:, :TNP],
                       start=False, stop=(ci == 3))
                xo = wp.tile([128, TNP], F32, name=f"xo{it}_{l}_{km}",
                             tag=f"xo{l}{km}")
                nc.vector.tensor_scalar(_r(xo[:, :TNP]), px[:, :],
                                        b2c[:, l * 2 + km:l * 2 + km + 1],
                                        None, OP.add)
                xnext.append(xo)
            return xnext

        for it in range(NT):
            T0 = it * TN
            xf = [wp.tile([128, TNP], F32, name=f"xf{it}_{h}", tag=f"xf{h}",
                          bufs=3) for h in range(2)]
            for h in range(2):
                nc.sync.dma_start(_r(xf[h][:, :]),
                                  _r(xfm[h * 128:(h + 1) * 128, T0:T0 + TNP]))

            # ---- proj + ELU -> h_route (token-major per slot, bf16)
            groups = [(0, (0, 1)), (1, (2, 3)), (2, (4,))]
            hr = {}
            for g, slots in (groups if on("proj") else []):
                gw = H * len(slots)
                pz = pp.tile([NB, 512], F32, name=f"pz{it}_{g}", tag="A",
                             bufs=3)
                for si, j in enumerate(slots):
                    tok = j * NB
                    zs = pz[:, si * H:(si + 1) * H]
                    mm(zs, _r(xf[0][:, tok:tok + NB]), _r(pW_sb[0][:, :]),
                       start=True, stop=False)
                    mm(zs, _r(xf[1][:, tok:tok + NB]), _r(pW_sb[1][:, :]),
                       start=False, stop=not pb_nz)
                    if pb_nz:
                        mm(zs, _r(onesr[:, :NB]), _r(pb_sb[:, :]),
                           start=False, stop=True)
                e = wp.tile([NB, 512], BF16, name=f"pe{it}_{g}", tag="pe")
                if not on("projelu"):
                    for si, j in enumerate(slots):
                        hr[j] = e[:, si * H:(si + 1) * H]
                    continue
                nc.scalar.activation(e[:, :gw], pz[:, :gw], AF.Exp)
                em1 = wp.tile([NB, 512], BF16, name=f"pem{it}_{g}", tag="pem")
                nc.vector.tensor_scalar(em1[:, :gw], e[:, :gw], -1.0, None,
                                        OP.add)
                h = wp.tile([NB, 512], BF16, name=f"hr{it}_{g}", tag=f"hr{g}")
                nc.vector.scalar_tensor_tensor(h[:, :gw], pz[:, :gw], 0.0,
                                               em1[:, :gw], OP.max, OP.min)
                for si, j in enumerate(slots):
                    hr[j] = h[:, si * H:(si + 1) * H]

            # ---- messages (feature-major out): m_fm[h,d] over slots
            pmf = [pp.tile([128, TNP], F32, name=f"pmf{it}_{h}", tag="B",
                           bufs=4) for h in range(2)]
            for h in (range(2) if on("memset") else []):
                nc.vector.memset(pmf[h][:, TN:TNP], 0.0)
            for j in (range(TB) if on("msg") else []):
                u = int(slot_type[it * TB + j])
                at = AT_sb[:, u * NB:(u + 1) * NB]
                for h in range(2):
                    mm(pmf[h][:, j * NB:(j + 1) * NB],
                       hr[j][:, h * 128:(h + 1) * 128], at)
            m_fm = [wp.tile([128, TNP], BF16, name=f"mf{it}_{h}", tag=f"mf{h}")
                    for h in range(2)]
            for h in range(2):
                nc.vector.tensor_copy(m_fm[h][:, :TNP], pmf[h][:, :])

            # ---- gate
            pg = [pp.tile([128, TNP], F32, name=f"pg{it}_{km}", tag="A",
                          bufs=3) for km in range(2)]
            for km in (range(2) if on("gate") else []):
                ks = slice(km * 128, (km + 1) * 128)
                mm(pg[km][:, :], _r(gWx_sb[0][:, ks]), _r(xf[0][:, :TNP]),
                   start=True, stop=False)
                mm(pg[km][:, :], _r(gWx_sb[1][:, ks]), _r(xf[1][:, :TNP]),
                   start=False, stop=not on("gatebf"))
                if on("gatebf"):
                    mm(pg[km][:, :], gWm_sb[0][:, ks], m_fm[0][:, :TNP],
                       start=False, stop=False)
                    mm(pg[km][:, :], gWm_sb[1][:, ks], m_fm[1][:, :TNP],
                       start=False, stop=True)
            # sigma = 1/(1+exp(-(z+gb)))
            x1 = []
            for km in range(2):
                if not on("sigma"):
                    x1.append(xf[km]); continue
                u = wp.tile([128, TNP], F32, name=f"u{it}_{km}", tag=f"u{km}")
                nc.scalar.activation(u[:, :TNP], pg[km][:, :], AF.Exp,
                                     bias=gbn[:, km:km + 1], scale=-1.0)
                w1p = wp.tile([128, TNP], F32, name=f"w1p{it}_{km}",
                              tag=f"w1p{km}")
                nc.vector.tensor_scalar(w1p[:, :], u[:, :], 1.0, None, OP.add)
                r = wp.tile([128, TNP], F32, name=f"r{it}_{km}", tag=f"r{km}")
                nc.vector.reciprocal(r[:, :TNP], w1p[:, :TNP])
                t1 = wp.tile([128, TNP], BF16, name=f"t1{it}_{km}",
                             tag=f"t1{km}")
                nc.gpsimd.tensor_tensor(t1[:, :TNP], r[:, :TNP],
                                        m_fm[km][:, :TNP], OP.mult)
                xx = wp.tile([128, TNP], F32, name=f"x1{it}_{km}",
                             tag=f"x1{km}")
                nc.gpsimd.tensor_tensor(_r(xx[:, :TNP]), xf[km][:, :TNP],
                                        t1[:, :TNP], OP.add)
                x1.append(xx)

            # ---- FFN stack
            if on("ffn"):
                x2 = layernorm_and_ffn(it, 0, x1)
                x3 = layernorm_and_ffn(it, 1, x2)
            else:
                x3 = x1
            for h in range(2):
                nc.sync.dma_start(xout[h * 128:(h + 1) * 128, T0:T0 + TN],
                                  x3[h][:, :TN])

    nc.compile()
    return nc


# --------------------------------------------------------------------------
# Host packing / unpacking
# --------------------------------------------------------------------------

def _host_params(proj_W, proj_b, gate_W, gate_b, ln_g, ln_b, W1, b1, W2, b2):
    W1g = W1 * ln_g[:, :, None]                      # (NL, H, K1)
    negc1 = -(W1g.sum(axis=1))                       # (NL, K1)
    d1 = np.einsum("lhk,lh->lk", W1, ln_b) + b1      # (NL, K1)
    return dict(
        pW=proj_W.astype(np.float32), proj_b=proj_b.astype(np.float32),
        gWx=gate_W[:H].astype(np.float32), gWm=gate_W[H:].astype(np.float32),
        gbn=(-gate_b).reshape(2, 128).T.copy(),
        W1g=W1g.astype(np.float32), negc1=negc1.astype(np.float32),
        d1=d1.astype(np.float32), W2=W2.astype(np.float32),
        b2c=np.stack([b2[l].reshape(2, 128).T for l in range(NL)], axis=1)
            .reshape(128, NL * 2),
    )


def _in_map_for_core(core, plan, params, block_tokens):
    Q = plan["Q"]
    TT = Q * NB
    bs = plan["per_core_b"][core]
    xs = block_tokens[bs]                        # (Q, NB, H)
    xfm = np.zeros((H, TT + TNP), np.float32)
    xfm[:, :TT] = xs.reshape(Q * NB, H).T        # (H, TT) + zero pad
    nun = plan["nun"]
    ATb = np.ascontiguousarray(
        plan["AT"].transpose(1, 0, 2).reshape(NB, nun * NB)).astype(BF16_NP)
    m = dict(
        xfm=xfm.astype(np.float32),
        ATm=ATb,
        wvd=plan["wvec"].astype(np.float32),
        pWd=params["pW"],
        gWxd=params["gWx"],
        gWmd=params["gWm"].astype(BF16_NP),
        W1gd=params["W1g"],
        W2d=params["W2"].astype(BF16_NP),
        negc1d=params["negc1"].reshape(1, NL * K1),
        gbnd=params["gbn"].astype(np.float32),
        b2cd=params["b2c"].astype(np.float32),
        identd=np.eye(128, dtype=np.float32),
        onescd=np.ones((128, 1), np.float32),
        onesrd=np.ones((1, 128), np.float32),
    )
    if np.any(params["proj_b"] != 0.0):
        m["pbd"] = params["proj_b"].reshape(1, H)
    if np.any(params["d1"] != 0.0):
        m["d1d"] = params["d1"].reshape(1, NL * K1)
        m["onestnd"] = np.ones((1, TNP), np.float32)
    return m


_CACHE = {}


def kernel(block_tokens, cancer_type, block_active, edge_src, edge_dst,
           edge_structure, ct_w, proj_W, proj_b, gate_W, gate_b,
           ln_g, ln_b, W1, b1, W2, b2):
    block_tokens = np.asarray(block_tokens, np.float32)
    cancer_type = np.asarray(cancer_type, np.int32)
    block_active = np.asarray(block_active, bool)
    edge_src = np.asarray(edge_src, np.int32)
    edge_dst = np.asarray(edge_dst, np.int32)

    plan = _plan_slots(cancer_type, block_active, edge_src, edge_dst,
                       np.asarray(edge_structure, np.float32),
                       np.asarray(ct_w, np.float32))
    params = _host_params(np.asarray(proj_W, np.float32),
                          np.asarray(proj_b, np.float32),
                          np.asarray(gate_W, np.float32),
                          np.asarray(gate_b, np.float32),
                          np.asarray(ln_g, np.float32),
                          np.asarray(ln_b, np.float32),
                          np.asarray(W1, np.float32),
                          np.asarray(b1, np.float32),
                          np.asarray(W2, np.float32),
                          np.asarray(b2, np.float32))

    key = (plan["Q"], plan["nun"], tuple(plan["slot_type"].tolist()),
           bool(np.any(params["proj_b"] != 0)), bool(np.any(params["d1"] != 0)))
    if key not in _CACHE:
        _CACHE.clear()
        _CACHE[key] = build_program(plan, params)
    nc = _CACHE[key]

    in_maps = [_in_map_for_core(i, plan, params, block_tokens)
               for i in range(NCORES)]
    res = run_bass_kernel_spmd(nc, in_maps, list(range(NCORES)))

    return _unshard(plan, res.results, block_tokens.shape[0])


def _unshard(plan, results, n_b):
    Q = plan["Q"]
    ncores = plan["per_core_b"].shape[0]
    x_full = np.zeros((n_b, NB, H), np.float32)
    ew_full = np.zeros((n_b, E), np.float32)
    for core in range(ncores):
        xo = results[core]["xout"]              # (H, Q*NB)
        eo = results[core]["ewout"]             # (Q, E)
        xr = xo.reshape(H, Q, NB).transpose(1, 2, 0)   # (Q, NB, H)
        bs = plan["per_core_b"][core]
        # later (duplicate-padded) slots may overwrite with identical data
        x_full[bs] = xr
        ew_full[bs] = eo
    return x_full, ew_full
